# revision 2
# baseline (speedup 1.0000x reference)
import numpy as np
import jax
import jax.numpy as jnp
from functools import partial

# nn_PointerNet: B=65536 samples, MAX_OBJECTS=20, NODE_DIM=16, H=64.
# Pure data parallel across 8 NeuronCores: shard B, replicate the tiny weights.
B = 65536
N = 20
D = 16
H = 64
NCORES = 8
NEG = -1.0e9
EPS = 1e-5

_WNAMES = [
    "w_embed", "b_embed", "w_mp1a", "b_mp1a", "w_mp1b", "b_mp1b",
    "w_mp2a", "b_mp2a", "w_mp2b", "b_mp2b", "g1", "be1", "g2", "be2",
    "w_pool", "b_pool", "w_op", "b_op", "w_c1", "b_c1", "w_c2", "b_c2",
    "w_q", "b_q", "w_k", "b_k",
]


def _layernorm(x, g, b):
    m = x.mean(-1, keepdims=True)
    v = jnp.mean((x - m) ** 2, axis=-1, keepdims=True)
    return (x - m) / jnp.sqrt(v + EPS) * g + b


def _forward(nf, nn_count, w):
    (w_embed, b_embed, w_mp1a, b_mp1a, w_mp1b, b_mp1b,
     w_mp2a, b_mp2a, w_mp2b, b_mp2b, g1, be1, g2, be2,
     w_pool, b_pool, w_op, b_op, w_c1, b_c1, w_c2, b_c2,
     w_q, b_q, w_k, b_k) = w
    mask = jnp.arange(N)[None, :] < nn_count[:, None]
    mf = mask.astype(jnp.float32)[..., None]
    denom = jnp.maximum(mf.sum(1, keepdims=True), 1.0)

    h = nf @ w_embed + b_embed
    msg = (h * mf).sum(1, keepdims=True) / denom
    z = h @ w_mp1a[:H] + msg @ w_mp1a[H:] + b_mp1a
    h = h + jax.nn.relu(z) @ w_mp1b + b_mp1b
    h = _layernorm(h, g1, be1) * mf

    msg = (h * mf).sum(1, keepdims=True) / denom
    z = h @ w_mp2a[:H] + msg @ w_mp2a[H:] + b_mp2a
    h = h + jax.nn.relu(z) @ w_mp2b + b_mp2b
    h = _layernorm(h, g2, be2) * mf

    pooled = (h * mf).sum(1) / jnp.maximum(mf.sum(1), 1.0)
    g = jax.nn.relu(pooled @ w_pool + b_pool)

    q = g @ w_q + b_q
    k = h @ w_k + b_k
    ptr_l = jnp.einsum('bh,bnh->bn', q, k)
    ptr_l = jnp.where(mask, ptr_l, NEG)
    return (g @ w_op + b_op, g @ w_c1 + b_c1, g @ w_c2 + b_c2, ptr_l)


_PFN = None


def _get_pfn():
    global _PFN
    if _PFN is None:
        _PFN = jax.pmap(_forward, in_axes=(0, 0, None), axis_name="x")
    return _PFN


def kernel(**inputs):
    nf = np.asarray(inputs["nf"], dtype=np.float32).reshape(NCORES, B // NCORES, N, D)
    cnt = np.asarray(inputs["nn_count"]).astype(np.int32).reshape(NCORES, B // NCORES)
    w = tuple(jnp.asarray(np.asarray(inputs[k], dtype=np.float32)) for k in _WNAMES)
    out = _get_pfn()(jnp.asarray(nf), jnp.asarray(cnt), w)
    res = tuple(np.asarray(o).reshape(B, -1) for o in out)
    return (res[0], res[1], res[2], res[3].reshape(B, N))


# revision 3
# speedup vs baseline: 1.4694x; 1.4694x over previous
import numpy as np
import jax
import jax.numpy as jnp

# nn_PointerNet: B=65536 samples, MAX_OBJECTS=20, NODE_DIM=16, H=64.
# Pure data parallel across 8 NeuronCores: shard B, replicate the tiny weights.
# nf ships to the devices as fp16 (exact range fit, ~5e-4 quantization) to halve
# host->device bytes; all on-device math is fp32. The four outputs are packed
# into one [*, 48] array on device so the gather is a single transfer per core.
B = 65536
N = 20
D = 16
H = 64
NCORES = 8
NEG = -1.0e9
EPS = 1e-5

_WNAMES = [
    "w_embed", "b_embed", "w_mp1a", "b_mp1a", "w_mp1b", "b_mp1b",
    "w_mp2a", "b_mp2a", "w_mp2b", "b_mp2b", "g1", "be1", "g2", "be2",
    "w_pool", "b_pool", "w_op", "b_op", "w_c1", "b_c1", "w_c2", "b_c2",
    "w_q", "b_q", "w_k", "b_k",
]


def _layernorm(x, g, b):
    m = x.mean(-1, keepdims=True)
    v = jnp.mean((x - m) ** 2, axis=-1, keepdims=True)
    return (x - m) / jnp.sqrt(v + EPS) * g + b


def _forward(nf16, nn_count, w):
    (w_embed, b_embed, w_mp1a, b_mp1a, w_mp1b, b_mp1b,
     w_mp2a, b_mp2a, w_mp2b, b_mp2b, g1, be1, g2, be2,
     w_pool, b_pool, w_op, b_op, w_c1, b_c1, w_c2, b_c2,
     w_q, b_q, w_k, b_k) = w
    nf = nf16.astype(jnp.float32)
    mask = jnp.arange(N)[None, :] < nn_count[:, None]
    mf = mask.astype(jnp.float32)[..., None]
    denom = jnp.maximum(mf.sum(1, keepdims=True), 1.0)

    h = nf @ w_embed + b_embed
    msg = (h * mf).sum(1, keepdims=True) / denom
    z = h @ w_mp1a[:H] + msg @ w_mp1a[H:] + b_mp1a
    h = h + jax.nn.relu(z) @ w_mp1b + b_mp1b
    h = _layernorm(h, g1, be1) * mf

    msg = (h * mf).sum(1, keepdims=True) / denom
    z = h @ w_mp2a[:H] + msg @ w_mp2a[H:] + b_mp2a
    h = h + jax.nn.relu(z) @ w_mp2b + b_mp2b
    h = _layernorm(h, g2, be2) * mf

    pooled = (h * mf).sum(1) / jnp.maximum(mf.sum(1), 1.0)
    g = jax.nn.relu(pooled @ w_pool + b_pool)

    q = g @ w_q + b_q
    k = h @ w_k + b_k
    ptr_l = (k * q[:, None, :]).sum(-1)
    ptr_l = jnp.where(mask, ptr_l, NEG)
    return jnp.concatenate(
        [g @ w_op + b_op, g @ w_c1 + b_c1, g @ w_c2 + b_c2, ptr_l], axis=-1
    )


_PFN = None


def _get_pfn():
    global _PFN
    if _PFN is None:
        _PFN = jax.pmap(_forward, in_axes=(0, 0, None), axis_name="x")
    return _PFN


def kernel(**inputs):
    nf16 = np.asarray(inputs["nf"], dtype=np.float32).astype(np.float16)
    nf16 = nf16.reshape(NCORES, B // NCORES, N, D)
    cnt = np.asarray(inputs["nn_count"]).astype(np.int32).reshape(NCORES, B // NCORES)
    w = tuple(jnp.asarray(np.asarray(inputs[k], dtype=np.float32)) for k in _WNAMES)
    out = _get_pfn()(jnp.asarray(nf16), jnp.asarray(cnt), w)
    packed = np.asarray(out).reshape(B, 48)
    return (
        np.ascontiguousarray(packed[:, 0:8]),
        np.ascontiguousarray(packed[:, 8:18]),
        np.ascontiguousarray(packed[:, 18:28]),
        np.ascontiguousarray(packed[:, 28:48]),
    )


# revision 5
# speedup vs baseline: 1.9169x; 1.3045x over previous
import numpy as np
import jax
import jax.numpy as jnp

# nn_PointerNet: B=65536 samples, MAX_OBJECTS=20, NODE_DIM=16, H=64.
# Pure data parallel across 8 NeuronCores: shard B, replicate the tiny weights.
# nf ships to the devices as fp16 (exact range fit, ~5e-4 quantization) to halve
# host->device bytes; all on-device math is fp32. The four outputs are packed
# into one [*, 48] array on device so the gather is a single transfer per core.
B = 65536
N = 20
D = 16
H = 64
NCORES = 8
NEG = -1.0e9
EPS = 1e-5

_WNAMES = [
    "w_embed", "b_embed", "w_mp1a", "b_mp1a", "w_mp1b", "b_mp1b",
    "w_mp2a", "b_mp2a", "w_mp2b", "b_mp2b", "g1", "be1", "g2", "be2",
    "w_pool", "b_pool", "w_op", "b_op", "w_c1", "b_c1", "w_c2", "b_c2",
    "w_q", "b_q", "w_k", "b_k",
]


def _layernorm(x, g, b):
    m = x.mean(-1, keepdims=True)
    v = jnp.mean((x - m) ** 2, axis=-1, keepdims=True)
    return (x - m) / jnp.sqrt(v + EPS) * g + b


def _forward(nf16, nn_count, w):
    (w_embed, b_embed, w_mp1a, b_mp1a, w_mp1b, b_mp1b,
     w_mp2a, b_mp2a, w_mp2b, b_mp2b, g1, be1, g2, be2,
     w_pool, b_pool, w_op, b_op, w_c1, b_c1, w_c2, b_c2,
     w_q, b_q, w_k, b_k) = w
    nf = nf16.astype(jnp.float32)
    mask = jnp.arange(N)[None, :] < nn_count[:, None]
    mf = mask.astype(jnp.float32)[..., None]
    denom = jnp.maximum(mf.sum(1, keepdims=True), 1.0)

    h = nf @ w_embed + b_embed
    msg = (h * mf).sum(1, keepdims=True) / denom
    z = h @ w_mp1a[:H] + msg @ w_mp1a[H:] + b_mp1a
    h = h + jax.nn.relu(z) @ w_mp1b + b_mp1b
    h = _layernorm(h, g1, be1) * mf

    msg = (h * mf).sum(1, keepdims=True) / denom
    z = h @ w_mp2a[:H] + msg @ w_mp2a[H:] + b_mp2a
    h = h + jax.nn.relu(z) @ w_mp2b + b_mp2b
    h = _layernorm(h, g2, be2) * mf

    pooled = (h * mf).sum(1) / jnp.maximum(mf.sum(1), 1.0)
    g = jax.nn.relu(pooled @ w_pool + b_pool)

    q = g @ w_q + b_q
    k = h @ w_k + b_k
    ptr_l = (k * q[:, None, :]).sum(-1)
    return jnp.concatenate(
        [g @ w_op + b_op, g @ w_c1 + b_c1, g @ w_c2 + b_c2, ptr_l], axis=-1
    ).astype(jnp.float16)


_PFN = None


def _get_pfn():
    global _PFN
    if _PFN is None:
        _PFN = jax.pmap(_forward, in_axes=(0, 0, None), axis_name="x")
    return _PFN


def kernel(**inputs):
    nf16 = np.asarray(inputs["nf"], dtype=np.float32).astype(np.float16)
    nf16 = nf16.reshape(NCORES, B // NCORES, N, D)
    cnt = np.asarray(inputs["nn_count"]).astype(np.int32).reshape(NCORES, B // NCORES)
    w = tuple(jnp.asarray(np.asarray(inputs[k], dtype=np.float32)) for k in _WNAMES)
    out = _get_pfn()(jnp.asarray(nf16), jnp.asarray(cnt), w)
    packed = np.asarray(out).reshape(B, 48).astype(np.float32)
    ptr = np.ascontiguousarray(packed[:, 28:48])
    invalid = np.arange(N)[None, :] >= cnt.reshape(B)[:, None]
    ptr[invalid] = NEG
    return (
        np.ascontiguousarray(packed[:, 0:8]),
        np.ascontiguousarray(packed[:, 8:18]),
        np.ascontiguousarray(packed[:, 18:28]),
        ptr,
    )


# revision 6
# speedup vs baseline: 2.2068x; 1.1513x over previous
import numpy as np
import jax
import jax.numpy as jnp

# nn_PointerNet: B=65536 samples, MAX_OBJECTS=20, NODE_DIM=16, H=64.
# Pure data parallel across 8 NeuronCores: shard B, replicate the tiny weights.
# nf ships to the devices as fp16 (exact range fit, ~5e-4 quantization) to halve
# host->device bytes; all on-device math is fp32. The four outputs are packed
# into one [*, 48] array on device so the gather is a single transfer per core.
B = 65536
N = 20
D = 16
H = 64
NCORES = 8
NEG = -1.0e9
EPS = 1e-5

_WNAMES = [
    "w_embed", "b_embed", "w_mp1a", "b_mp1a", "w_mp1b", "b_mp1b",
    "w_mp2a", "b_mp2a", "w_mp2b", "b_mp2b", "g1", "be1", "g2", "be2",
    "w_pool", "b_pool", "w_op", "b_op", "w_c1", "b_c1", "w_c2", "b_c2",
    "w_q", "b_q", "w_k", "b_k",
]


def _layernorm(x, g, b):
    m = x.mean(-1, keepdims=True)
    v = jnp.mean((x - m) ** 2, axis=-1, keepdims=True)
    return (x - m) / jnp.sqrt(v + EPS) * g + b


def _forward(nf16, nn_count, w):
    (w_embed, b_embed, w_mp1a, b_mp1a, w_mp1b, b_mp1b,
     w_mp2a, b_mp2a, w_mp2b, b_mp2b, g1, be1, g2, be2,
     w_pool, b_pool, w_op, b_op, w_c1, b_c1, w_c2, b_c2,
     w_q, b_q, w_k, b_k) = w
    nf = nf16.astype(jnp.float32)
    mask = jnp.arange(N)[None, :] < nn_count[:, None]
    mf = mask.astype(jnp.float32)[..., None]
    denom = jnp.maximum(mf.sum(1, keepdims=True), 1.0)

    h = nf @ w_embed + b_embed
    msg = (h * mf).sum(1, keepdims=True) / denom
    z = h @ w_mp1a[:H] + msg @ w_mp1a[H:] + b_mp1a
    h = h + jax.nn.relu(z) @ w_mp1b + b_mp1b
    h = _layernorm(h, g1, be1) * mf

    msg = (h * mf).sum(1, keepdims=True) / denom
    z = h @ w_mp2a[:H] + msg @ w_mp2a[H:] + b_mp2a
    h = h + jax.nn.relu(z) @ w_mp2b + b_mp2b
    h = _layernorm(h, g2, be2) * mf

    pooled = (h * mf).sum(1) / jnp.maximum(mf.sum(1), 1.0)
    g = jax.nn.relu(pooled @ w_pool + b_pool)

    q = g @ w_q + b_q
    k = h @ w_k + b_k
    ptr_l = (k * q[:, None, :]).sum(-1)
    return jnp.concatenate(
        [g @ w_op + b_op, g @ w_c1 + b_c1, g @ w_c2 + b_c2, ptr_l], axis=-1
    ).astype(jnp.float16)


_PFN = None


def _get_pfn():
    global _PFN
    if _PFN is None:
        _PFN = jax.pmap(_forward, in_axes=(0, 0, None), axis_name="x")
    return _PFN


def kernel(**inputs):
    nf16 = np.asarray(inputs["nf"], dtype=np.float32).astype(np.float16)
    nf16 = nf16.reshape(NCORES, B // NCORES, N, D)
    cnt = np.asarray(inputs["nn_count"]).astype(np.int32).reshape(NCORES, B // NCORES)
    w = tuple(jnp.asarray(np.asarray(inputs[k], dtype=np.float32)) for k in _WNAMES)
    devs = jax.devices()[:NCORES]
    try:
        dnf = jax.device_put_sharded([nf16[i] for i in range(NCORES)], devs)
        dcnt = jax.device_put_sharded([cnt[i] for i in range(NCORES)], devs)
    except Exception:
        dnf, dcnt = jnp.asarray(nf16), jnp.asarray(cnt)
    out = _get_pfn()(dnf, dcnt, w)
    packed = np.asarray(out).reshape(B, 48).astype(np.float32)
    ptr = np.ascontiguousarray(packed[:, 28:48])
    invalid = np.arange(N)[None, :] >= cnt.reshape(B)[:, None]
    ptr[invalid] = NEG
    return (
        np.ascontiguousarray(packed[:, 0:8]),
        np.ascontiguousarray(packed[:, 8:18]),
        np.ascontiguousarray(packed[:, 18:28]),
        ptr,
    )
